# revision 1
# baseline (speedup 1.0000x reference)
"""Trainium2 Bass kernel for nn_CustomQuantumLayer (4-qubit circuit, B=1048576).

Self-contained: computes evs = Z-expectations of U @ (kron of per-sample RY
states) for a fixed entangling unitary U built host-side from `weights`.

Distribution: pure data parallelism over 8 NeuronCores (batch sharded 8 ways,
tiny weight-derived matrices replicated).

Per-core pipeline (N=131072 samples, 2 halves of 65536, T=512 samples/partition):
  trig (ACT Sin)  -> c/s planes [128, 8, 512] fp16
  psi build (DVE/GPSIMD muls) -> psi [128, 16, 512] fp16
  PE transpose    -> psiT tiles [128 = 8u+r, 512 = 128dd+p]
  M1 (PE, fp16)   -> phi2 = [Re U; Im U] @ psi, 32 comps/sample, PSUM
  square (DVE/ACT)-> sq fp16 SBUF
  M2 (PE, fp16, accumulating) -> evs [128 = 32a2+16a1+4rr+q, 512] PSUM
  copy + PE transpose back -> [p, (dd, m)] -> DMA straight to DRAM rows [b, 4]
Sample index: b = h*65536 + 512p + 128dd + 32tg + 8a2 + 4a1 + rr, out elem
addr = 4b + q = 262144h + 2048p + 512dd + 128tg + m with m = 32a2+16a1+4rr+q.
"""
import math

import numpy as np

N_QUBITS = 4
RANGES = [1, 2]
B_TOTAL = 1048576
N_CORES = 8
N_CORE = B_TOTAL // N_CORES  # 131072
N_HALF = N_CORE // 2  # 65536
T = N_HALF // 128  # 512

_CACHE = {}


# ---------------------------------------------------------------- host math
def _build_u(weights):
    w = np.asarray(weights, np.float64)

    def ry(t):
        c, s = np.cos(t / 2), np.sin(t / 2)
        return np.array([[c, -s], [s, c]], np.complex128)

    def rz(t):
        e = np.exp(-0.5j * t)
        return np.array([[e, 0], [0, np.conj(e)]], np.complex128)

    def rot(phi, th, om):
        return rz(om) @ ry(th) @ rz(phi)

    def emb1(g, q):
        m = np.array([[1.0]], np.complex128)
        for k in range(N_QUBITS):
            m = np.kron(m, g if k == q else np.eye(2, dtype=np.complex128))
        return m

    def cnot(c, t):
        m = np.zeros((16, 16), np.complex128)
        for s in range(16):
            bits = [(s >> (3 - k)) & 1 for k in range(N_QUBITS)]
            if bits[c] == 1:
                bits[t] ^= 1
            s2 = sum(b << (3 - k) for k, b in enumerate(bits))
            m[s2, s] = 1.0
        return m

    U = np.eye(16, dtype=np.complex128)
    for l in range(2):
        for q in range(N_QUBITS):
            U = emb1(rot(*w[l, q]), q) @ U
        for q in range(N_QUBITS):
            U = cnot(q, (q + RANGES[l]) % N_QUBITS) @ U
    return U


def _build_consts(weights):
    U = _build_u(weights)
    W32 = np.concatenate([U.real, U.imag], axis=0)  # [32, 16]

    Zs = np.zeros((32, 4), np.float64)
    for cc in range(32):
        s = cc % 16
        for q in range(4):
            Zs[cc, q] = 1.0 - 2.0 * ((s >> (3 - q)) & 1)

    lhsT_A = np.zeros((128, 128), np.float64)
    lhsT_B = np.zeros((128, 128), np.float64)
    for u in range(16):
        for r in range(8):
            for cc in range(32):
                if r < 4:
                    lhsT_A[8 * u + r, 32 * r + cc] = W32[cc, u]
                else:
                    lhsT_B[8 * u + r, 32 * (r - 4) + cc] = W32[cc, u]

    lhsT_Z = np.zeros((8, 128, 128), np.float64)
    for i in range(8):
        for rr in range(4):
            for cc in range(32):
                for q in range(4):
                    lhsT_Z[i, 32 * rr + cc, 16 * i + 4 * rr + q] = Zs[cc, q]
    return lhsT_A, lhsT_B, lhsT_Z


# ---------------------------------------------------------------- device kernel
def _emit_kernel(tc, outs, ins):
    from contextlib import ExitStack

    import concourse.bass as bass
    import concourse.mybir as mybir

    ctx = ExitStack()

    nc = tc.nc
    f32 = mybir.dt.float32
    f16 = mybir.dt.float16
    Act = mybir.ActivationFunctionType

    x_ap = ins["x"]
    la_ap = ins["lhsT_A"]
    lb_ap = ins["lhsT_B"]
    lz_ap = ins["lhsT_Z"]
    id16_ap = ins["ident16"]
    id32_ap = ins["ident32"]
    out_ap = outs["out"]

    consts = ctx.enter_context(tc.tile_pool(name="consts", bufs=1))
    sb_x = ctx.enter_context(tc.tile_pool(name="x", bufs=2))
    sb_cs = ctx.enter_context(tc.tile_pool(name="cs", bufs=2))
    sb_t = ctx.enter_context(tc.tile_pool(name="t01", bufs=2))
    sb_psi = ctx.enter_context(tc.tile_pool(name="psi", bufs=2))
    sb_psiT = ctx.enter_context(tc.tile_pool(name="psiT", bufs=6))
    sb_sq = ctx.enter_context(tc.tile_pool(name="sq", bufs=6))
    sb_evs = ctx.enter_context(tc.tile_pool(name="evs", bufs=4))
    ps_tr = ctx.enter_context(tc.tile_pool(name="ps_tr", bufs=2, space="PSUM"))
    ps_phi = ctx.enter_context(tc.tile_pool(name="ps_phi", bufs=2, space="PSUM"))
    ps_evs = ctx.enter_context(tc.tile_pool(name="ps_evs", bufs=1, space="PSUM"))
    ps_evsT = ctx.enter_context(tc.tile_pool(name="ps_evsT", bufs=1, space="PSUM"))

    # constants
    lA = consts.tile([128, 128], f16, tag="lA")
    lB = consts.tile([128, 128], f16, tag="lB")
    lZ = consts.tile([128, 8, 128], f16, tag="lZ")
    id16 = consts.tile([128, 128], f16, tag="id16")
    id32 = consts.tile([128, 128], f32, tag="id32")
    nc.sync.dma_start(out=lA, in_=la_ap)
    nc.sync.dma_start(out=lB, in_=lb_ap)
    nc.sync.dma_start(out=lZ, in_=lz_ap.rearrange("i k m -> k i m"))
    nc.sync.dma_start(out=id16, in_=id16_ap)
    nc.sync.dma_start(out=id32, in_=id32_ap)

    for h in range(2):
        x_t = sb_x.tile([128, T, 4], f32, tag="x")
        nc.sync.dma_start(
            out=x_t,
            in_=x_ap[h * N_HALF:(h + 1) * N_HALF, :].rearrange(
                "(p t) f -> p t f", p=128),
        )
        # trig. ACT Sin domain is [-pi, pi]: sin(x/2) is safe for |x| < 2*pi;
        # cos(x/2) = 1 - 2*sin(x/4)^2 keeps the argument in [-pi/2, pi/2].
        # Contiguous full-tile Sin reads (strided ACT reads are 2x slower);
        # outputs stay interleaved [t, k].
        sins = sb_cs.tile([128, T, 4], f16, tag="sins")
        sinq = sb_cs.tile([128, T, 4], f16, tag="sinq")
        w2 = sb_cs.tile([128, T, 4], f16, tag="w2")
        c16 = sb_cs.tile([128, T, 4], f16, tag="c16")
        nc.scalar.activation(out=sins, in_=x_t, func=Act.Sin, scale=0.5)
        nc.scalar.activation(out=sinq, in_=x_t, func=Act.Sin, scale=0.25)
        nc.vector.tensor_mul(out=w2, in0=sinq, in1=sinq)
        nc.vector.tensor_scalar(
            out=c16, in0=w2, scalar1=-2.0, scalar2=1.0,
            op0=mybir.AluOpType.mult, op1=mybir.AluOpType.add)

        def v(k, e):  # v_k[e]: e=0 -> cos(x_k/2), e=1 -> sin(x_k/2)
            src = c16 if e == 0 else sins
            return src[:, :, k]

        # psi build (all DVE: gpsimd tensor ops are ~4x slower)
        t01 = sb_t.tile([128, 4, T], f16, tag="t01")
        t23 = sb_t.tile([128, 4, T], f16, tag="t23")
        for a in range(2):
            for b in range(2):
                nc.vector.tensor_mul(
                    out=t01[:, 2 * a + b, :], in0=v(0, a), in1=v(1, b))
                nc.gpsimd.tensor_mul(
                    out=t23[:, 2 * a + b, :], in0=v(2, a), in1=v(3, b))
        # psi stored [128, j, u, r] so each transpose input [128, (u r)] is a
        # flat contiguous 128-wide free dim (matmul needs one free dim).
        psi = sb_psi.tile([128, T // 8, 16, 8], f16, tag="psi")
        for u in range(16):
            eng = nc.gpsimd if u % 4 == 3 else nc.vector
            eng.tensor_mul(
                out=psi[:, :, u, :],
                in0=t01[:, u // 4, :].rearrange("p (j r) -> p j r", r=8),
                in1=t23[:, u % 4, :].rearrange("p (j r) -> p j r", r=8))

        for tg in range(4):
            evs_ps = ps_evs.tile([128, 512], f32, tag="evs_ps")
            for a2 in range(4):
                ti = 4 * tg + a2
                tr_ps = ps_tr.tile([128, 512], f16, tag="tr")
                for dd in range(4):
                    j = ti + 16 * dd
                    nc.tensor.transpose(
                        out=tr_ps[:, 128 * dd:128 * (dd + 1)],
                        in_=psi[:, j, :, :].rearrange("p u r -> p (u r)"),
                        identity=id16,
                    )
                psiT = sb_psiT.tile([128, 512], f16, tag="psiT")
                nc.vector.tensor_copy(out=psiT, in_=tr_ps)
                phi_big = ps_phi.tile([128, 1024], f32, tag="phi")
                sq_big = sb_sq.tile([128, 1024], f16, tag="sq")
                nc.tensor.matmul(out=phi_big[:, 0:512], lhsT=lA,
                                 rhs=psiT, start=True, stop=True)
                nc.tensor.matmul(out=phi_big[:, 512:1024], lhsT=lB,
                                 rhs=psiT, start=True, stop=True)
                if a2 % 2 == 0:
                    nc.scalar.activation(out=sq_big, in_=phi_big, func=Act.Square)
                else:
                    sqf = sb_sq.tile([128, 1024], f16, tag="sqf")
                    nc.vector.tensor_copy(out=sqf, in_=phi_big)
                    nc.vector.tensor_mul(out=sq_big, in0=sqf, in1=sqf)
                nc.tensor.matmul(
                    out=evs_ps, lhsT=lZ[:, 2 * a2, :],
                    rhs=sq_big[:, 0:512],
                    start=(a2 == 0), stop=False)
                nc.tensor.matmul(
                    out=evs_ps, lhsT=lZ[:, 2 * a2 + 1, :],
                    rhs=sq_big[:, 512:1024],
                    start=False, stop=(a2 == 3))
            evsSB = sb_evs.tile([128, 512], f32, tag="evsSB")
            nc.scalar.activation(out=evsSB, in_=evs_ps, func=Act.Copy)
            evsT_ps = ps_evsT.tile([128, 512], f32, tag="evsT")
            for dd in range(4):
                nc.tensor.transpose(
                    out=evsT_ps[:, 128 * dd:128 * (dd + 1)],
                    in_=evsSB[:, 128 * dd:128 * (dd + 1)],
                    identity=id32,
                )
            evsT_sb = sb_evs.tile([128, 512], f32, tag="evsT_sb")
            nc.scalar.activation(out=evsT_sb, in_=evsT_ps, func=Act.Copy)
            dst = bass.AP(
                tensor=out_ap.tensor,
                offset=out_ap.offset + 262144 * h + 128 * tg,
                ap=[[2048, 128], [512, 4], [1, 128]],
            )
            nc.sync.dma_start(
                out=dst, in_=evsT_sb.rearrange("p (d m) -> p d m", d=4))

    ctx.close()


def _build_program():
    """Compile the SPMD program once; returns (nc, names)."""
    import concourse.bacc as bacc
    import concourse.mybir as mybir
    import concourse.tile as tile

    nc = bacc.Bacc("TRN2", debug=False, num_devices=N_CORES)
    f32 = mybir.dt.float32
    f16 = mybir.dt.float16

    ins = {
        "x": nc.dram_tensor("x", [N_CORE, 4], f32, kind="ExternalInput").ap(),
        "lhsT_A": nc.dram_tensor("lhsT_A", [128, 128], f16,
                                 kind="ExternalInput").ap(),
        "lhsT_B": nc.dram_tensor("lhsT_B", [128, 128], f16,
                                 kind="ExternalInput").ap(),
        "lhsT_Z": nc.dram_tensor("lhsT_Z", [8, 128, 128], f16,
                                 kind="ExternalInput").ap(),
        "ident16": nc.dram_tensor("ident16", [128, 128], f16,
                                  kind="ExternalInput").ap(),
        "ident32": nc.dram_tensor("ident32", [128, 128], f32,
                                  kind="ExternalInput").ap(),
    }
    outs = {
        "out": nc.dram_tensor("out", [N_CORE, 4], f32,
                              kind="ExternalOutput").ap(),
    }
    with tile.TileContext(nc) as tc:
        _emit_kernel(tc, outs, ins)
    nc.compile()
    return nc


def _get_program():
    if "nc" not in _CACHE:
        _CACHE["nc"] = _build_program()
    return _CACHE["nc"]


def kernel(x: np.ndarray, weights: np.ndarray) -> np.ndarray:
    from concourse import bass_utils

    nc = _get_program()
    lhsT_A, lhsT_B, lhsT_Z = _build_consts(weights)
    consts = {
        "lhsT_A": lhsT_A.astype(np.float16),
        "lhsT_B": lhsT_B.astype(np.float16),
        "lhsT_Z": lhsT_Z.astype(np.float16),
        "ident16": np.eye(128, dtype=np.float16),
        "ident32": np.eye(128, dtype=np.float32),
    }
    x = np.ascontiguousarray(np.asarray(x, np.float32))
    in_maps = []
    for c in range(N_CORES):
        m = {"x": x[c * N_CORE:(c + 1) * N_CORE]}
        m.update(consts)
        in_maps.append(m)
    res = bass_utils.run_bass_kernel_spmd(nc, in_maps, core_ids=list(range(N_CORES)))
    out = np.concatenate([res.results[c]["out"] for c in range(N_CORES)], axis=0)
    return out.astype(np.float32)

